# revision 39
# baseline (speedup 1.0000x reference)
"""Trainium2 Bass kernel for nn_Conv2d_NN_spatial (retrieval_knn).

Math (per batch):
  x1 = x.reshape(C, N); cand = x1[:, grid64]  (8x8 spatial grid, static)
  ranking key over candidates s for each position n:
      negkey[n, s] = 2*dot(x1[:,n], cand[:,s]) - ||cand[:,s]||^2
  (matches reference ordering incl. NaN rows; see prior analysis)
  top-3 indices i_k(n) (stable ties -> ascending index, = lax.top_k)
  out[:, n] = relu(b + sum_k W_k @ x1[:, i_k(n)])   (gather source is
   x1[:, 0:64], the reference's matrix[:, ind] quirk)
  selection via one-hot matmuls: out^T = sum_k Y'_k^T @ onehot_k

Precision scheme: x and candidates are split hi/lo into bf16 pairs on the
host (16-bit effective mantissa); the distance matmul streams
[2ch | 2cl] against stationary [xh ; xl], accumulating both halves into
the same PSUM columns via a stride-0 (broadcast) out AP, on top of a
-s2 prefill (rank-2 ones matmul with hi/lo s2 rows).  Measured end-to-end
rel err of this scheme ~4e-3 (gate 2e-2); ranking wrong-pick rate ~0.

Sharding: pure data parallel, 2 batches per core on 8 cores.
"""

import numpy as np

B, C, H, W = 16, 64, 96, 96
N = H * W            # 9216
S2 = 64              # candidate count (8x8 grid)
KNN = 3
NCORES = 8
BPC = B // NCORES    # batches per core = 2
NCHUNK = 128         # n per distance-matmul chunk
MACRO = 1024         # n per macro tile (8 chunks)
NMACRO = N // MACRO  # 9
PSC = 512            # psum score tile cols (4 chunks)

_CACHE = {}


def _build():
    import concourse.bass as bass
    import concourse.bacc as bacc
    import concourse.mybir as mybir
    import concourse.tile as tile
    from contextlib import ExitStack

    f32 = mybir.dt.float32
    bf16 = mybir.dt.bfloat16
    f16 = mybir.dt.float16
    u16 = mybir.dt.uint16
    AF = mybir.ActivationFunctionType

    nc = bacc.Bacc()
    xhl_h = nc.declare_dram_parameter("xhl", [BPC, 2 * C, N], bf16, isOutput=False)
    # consts f32: [0:128]=identity(f32, for idx transpose), 128=bias, 129=iotacol
    consts_h = nc.declare_dram_parameter("consts", [128, 130], f32, isOutput=False)
    # consts2 bf16: cols 0:192 = wstA ([Wh;Wh] per k), 192:384 = wstB
    # ([Wl;Wl]), 384:512 = rcand b0, 512:640 = rcand b1,
    # 640:1152 = s2rows (b0 rows 0:2, b1 rows 2:4)
    consts2_h = nc.declare_dram_parameter("consts2", [128, 1152], bf16, isOutput=False)
    out_h = nc.declare_dram_parameter("out", [BPC, C, N], f32, isOutput=True)

    with tile.TileContext(nc) as tc, ExitStack() as ctx:
        const_p = ctx.enter_context(tc.tile_pool(name="const", bufs=1))
        xin_p = ctx.enter_context(tc.tile_pool(name="xin", bufs=2))
        small_p = ctx.enter_context(tc.tile_pool(name="small", bufs=2))
        idx_p = ctx.enter_context(tc.tile_pool(name="idx", bufs=3))
        ssb_p = ctx.enter_context(tc.tile_pool(name="ssb", bufs=4))
        bcast_p = ctx.enter_context(tc.tile_pool(name="bcast", bufs=3))
        oh_p = ctx.enter_context(tc.tile_pool(name="oh", bufs=3))
        osb_p = ctx.enter_context(tc.tile_pool(name="osb", bufs=3))
        dram_p = ctx.enter_context(tc.tile_pool(name="bounce", bufs=3, space="DRAM"))
        ps_nk = ctx.enter_context(tc.tile_pool(name="ps_nk", bufs=3, space="PSUM"))
        ps_it = ctx.enter_context(tc.tile_pool(name="ps_it", bufs=1, space="PSUM"))
        ps_out = ctx.enter_context(tc.tile_pool(name="ps_out", bufs=1, space="PSUM"))
        ps_misc = ctx.enter_context(tc.tile_pool(name="ps_misc", bufs=1, space="PSUM"))

        consts_sb = const_p.tile([128, 130], f32)
        nc.sync.dma_start(consts_sb[:], consts_h.ap())
        identf_sb = consts_sb[:, 0:128]
        bias_sb = consts_sb[0:C, 128:129]
        iotacol_sb = consts_sb[:, 129:130]

        consts2_sb = const_p.tile([128, 1152], bf16)
        nc.sync.dma_start(consts2_sb[:], consts2_h.ap())

        ones2_sb = const_p.tile([66, 128], bf16, tag="ones2")
        nc.vector.memset(ones2_sb[:], 1.0)

        for b in range(BPC):
            # ---- x hi/lo [128, N] bf16 ----
            xhl = xin_p.tile([2 * C, N], bf16, tag="xhl")
            nc.sync.dma_start(xhl[:], xhl_h.ap()[b])

            rc_sb = consts2_sb[:, 384 + 128 * b : 384 + 128 * (b + 1)]
            s2row_sb = consts2_sb[64 * b : 64 * b + 2, 640:1152]
            ones2b_sb = ones2_sb[64 * b : 64 * b + 2, :]

            # ---- Y' tables: Y'_k[s,o] = sum_c x1[c, s]*W_k[o, c] ----
            # two passes: (xh+xl)Wh then += (xh+xl)Wl
            yall = ps_misc.tile([128, 128], f32, tag="misc")
            wstA = consts2_sb[:, 0:192]
            wstB = consts2_sb[:, 192:384]
            x64 = xhl[:, 0:S2]
            nc.tensor.matmul(yall[0:64, 0:64], x64, wstA[:, 0:64], start=True, stop=False)
            nc.tensor.matmul(yall[0:64, 0:64], x64, wstB[:, 0:64], start=False, stop=True)
            nc.tensor.matmul(yall[64:128, 0:64], x64, wstA[:, 64:128], start=True, stop=False)
            nc.tensor.matmul(yall[64:128, 0:64], x64, wstB[:, 64:128], start=False, stop=True)
            nc.tensor.matmul(yall[0:64, 64:128], x64, wstA[:, 128:192], start=True, stop=False)
            nc.tensor.matmul(yall[0:64, 64:128], x64, wstB[:, 128:192], start=False, stop=True)
            ysel01 = small_p.tile([128, C], f16, tag="ysel01")
            nc.vector.tensor_copy(ysel01[:], yall[:, 0:64])
            ysel2 = small_p.tile([C, C], f16, tag="ysel2")
            nc.vector.tensor_copy(ysel2[:], yall[0:64, 64:128])

            # ---- main loop over macro tiles (selection software-pipelined
            # one macro behind distance, to keep the in-order PE queue busy) ----
            def emit_selection(oh01, oh2, n0):
                po = ps_out.tile([64, MACRO], f32, tag="po")
                for h in range(2):
                    ph = po[:, h * 512 : (h + 1) * 512]
                    nc.tensor.matmul(ph, ysel01[:], oh01[:, h * 512 : (h + 1) * 512], start=True, stop=False)
                    nc.tensor.matmul(ph, ysel2[:], oh2[:, h * 512 : (h + 1) * 512], start=False, stop=True)
                osb = osb_p.tile([64, MACRO], f32, tag="osb")
                nc.scalar.activation(osb[:], po[:], AF.Relu, bias=bias_sb)
                nc.sync.dma_start(out_h.ap()[b][:, n0 : n0 + MACRO], osb[:])

            prev = None
            for m in range(NMACRO):
                n0 = m * MACRO
                idx_all = idx_p.tile([NCHUNK, 8, 8], u16, tag="idx")
                pnk = ps_nk.tile([NCHUNK, PSC], f32, tag="nk")
                # -s2 prefill of all 8 chunk-blocks (hi+lo rows)
                nc.tensor.matmul(pnk[:], ones2b_sb, s2row_sb[:], start=True, stop=False)
                for c8 in range(8):
                    sl = pnk[:, c8 * S2 : (c8 + 1) * S2]
                    xch = xhl[:, n0 + c8 * NCHUNK : n0 + (c8 + 1) * NCHUNK]
                    # (xh+xl)*2ch then += (xh+xl)*2cl, on top of -s2 prefill
                    nc.tensor.matmul(sl, xch, rc_sb[:, 0:S2], start=False, stop=False)
                    nc.tensor.matmul(sl, xch, rc_sb[:, S2:128], start=False, stop=True)
                if prev is not None:
                    emit_selection(*prev)
                for hh in range(2):
                    ssb = ssb_p.tile([NCHUNK, PSC // 2], f32, tag="ssb")
                    nc.scalar.copy(ssb[:], pnk[:, hh * 256 : (hh + 1) * 256])
                    for c4 in range(4):
                        c8 = hh * 4 + c4
                        ssl = ssb[:, c4 * S2 : (c4 + 1) * S2]
                        maxv = idx_p.tile([NCHUNK, 8], f32, tag="maxv")
                        nc.vector.max(out=maxv[:], in_=ssl)
                        nc.vector.max_index(out=idx_all[:, :, c8], in_max=maxv[:], in_values=ssl)

                # transpose idx (rank-major rows) via PE -> bounce -> bcast
                idxf = idx_p.tile([NCHUNK, 64], f32, tag="idxf")
                nc.vector.tensor_copy(idxf[:], idx_all[:].rearrange("p a b -> p (a b)"))
                pit = ps_it.tile([64, NCHUNK], f32, tag="it")
                nc.tensor.transpose(pit[:], idxf[:], identf_sb[:])
                idxt_sb = idx_p.tile([KNN * 8, NCHUNK], u16, tag="idxt_sb")
                nc.vector.tensor_copy(idxt_sb[:], pit[0 : KNN * 8, :])  # f32 -> u16
                dbt = dram_p.tile([KNN, MACRO], u16, tag="dbt")
                nc.sync.dma_start(dbt[:], idxt_sb[:])
                idxb01 = bcast_p.tile([128, MACRO], u16, tag="idxb01")
                idxb2 = bcast_p.tile([64, MACRO], u16, tag="idxb2")
                nc.sync.dma_start(idxb01[0:64, :], dbt[0:1, :].to_broadcast((64, MACRO)))
                nc.sync.dma_start(idxb01[64:128, :], dbt[1:2, :].to_broadcast((64, MACRO)))
                nc.sync.dma_start(idxb2[:], dbt[2:3, :].to_broadcast((64, MACRO)))

                # one-hots (fp16): oh[s, n] = [s == i_k(n)]
                oh01 = oh_p.tile([128, MACRO], f16, tag="oh01")
                oh2 = oh_p.tile([64, MACRO], f16, tag="oh2")
                nc.vector.tensor_scalar(
                    oh01[:], idxb01[:], iotacol_sb, None,
                    op0=mybir.AluOpType.is_equal,
                )
                nc.vector.tensor_scalar(
                    oh2[:], idxb2[:], iotacol_sb[0:64, :], None,
                    op0=mybir.AluOpType.is_equal,
                )

                prev = (oh01, oh2, n0)
            emit_selection(*prev)

    nc.compile()
    return nc


def _host_inputs(x, conv_w, conv_b):
    """Build all per-core DRAM inputs (host prep: dtype split + packing)."""
    import ml_dtypes
    bf = ml_dtypes.bfloat16

    def split(v):
        hi = v.astype(bf)
        lo = (v - hi.astype(np.float32)).astype(bf)
        return hi, lo

    xr = x.reshape(B, C, N)
    # sample grid (static): index s = 8i+j over rounded linspace
    xi = np.round(np.linspace(0, H - 1, 8)).astype(np.int32)
    yi = np.round(np.linspace(0, W - 1, 8)).astype(np.int32)
    xg, yg = np.meshgrid(xi, yi, indexing="ij")
    gidx = (xg * W + yg).reshape(-1)

    consts = np.zeros((128, 130), np.float32)
    consts[:, 0:128] = np.eye(128, dtype=np.float32)
    consts[0:C, 128] = conv_b
    consts[:, 129] = np.arange(128, dtype=np.float32) % 64

    wst = np.zeros((128, 384), bf)
    for k in range(KNN):
        wh, wl = split(conv_w[:, :, k].T)     # [c, o]
        wst[0:C, k * 64 : (k + 1) * 64] = wh
        wst[C:, k * 64 : (k + 1) * 64] = wh
        wst[0:C, 192 + k * 64 : 192 + (k + 1) * 64] = wl
        wst[C:, 192 + k * 64 : 192 + (k + 1) * 64] = wl

    in_maps = []
    for core in range(NCORES):
        consts2 = np.zeros((128, 1152), bf)
        consts2[:, 0:384] = wst
        xhl_core = np.empty((BPC, 2 * C, N), bf)
        for b in range(BPC):
            x1 = xr[core * BPC + b]          # [C, N] f32
            xh, xl = split(x1)
            xhl_core[b, 0:C] = xh
            xhl_core[b, C:] = xl
            cand = x1[:, gidx]               # [C, 64]
            ch, cl = split(2.0 * cand)
            base = 384 + 128 * b
            consts2[0:C, base : base + 64] = ch
            consts2[C:, base : base + 64] = ch
            consts2[0:C, base + 64 : base + 128] = cl
            consts2[C:, base + 64 : base + 128] = cl
            s2 = (cand.astype(np.float64) ** 2).sum(0).astype(np.float32)
            s2h, s2l = split(-s2)
            consts2[64 * b, 640:1152] = np.tile(s2h, 8)
            consts2[64 * b + 1, 640:1152] = np.tile(s2l, 8)
        in_maps.append({
            "xhl": xhl_core,
            "consts": consts,
            "consts2": consts2,
        })
    return in_maps


def kernel(x, conv_w, conv_b):
    from concourse.bass_utils import run_bass_kernel_spmd

    x = np.ascontiguousarray(np.asarray(x, dtype=np.float32))
    conv_w = np.asarray(conv_w, dtype=np.float32)
    conv_b = np.asarray(conv_b, dtype=np.float32)

    if "nc" not in _CACHE:
        _CACHE["nc"] = _build()
    nc = _CACHE["nc"]

    in_maps = _host_inputs(x, conv_w, conv_b)
    res = run_bass_kernel_spmd(nc, in_maps, list(range(NCORES))).results
    out = np.empty((B, C, N), np.float32)
    for core in range(NCORES):
        out[core * BPC : (core + 1) * BPC] = res[core]["out"].reshape(BPC, C, N)
    return out.reshape(B, C, H, W)


# revision 41
# speedup vs baseline: 1.1612x; 1.1612x over previous
"""Trainium2 Bass kernel for nn_Conv2d_NN_spatial (retrieval_knn).

Math (per batch):
  x1 = x.reshape(C, N); cand = x1[:, grid64]  (8x8 spatial grid, static)
  ranking key over candidates s for each position n:
      negkey[n, s] = 2*dot(x1[:,n], cand[:,s]) - ||cand[:,s]||^2
  (matches reference ordering incl. NaN rows; see prior analysis)
  top-3 indices i_k(n) (stable ties -> ascending index, = lax.top_k)
  out[:, n] = relu(b + sum_k W_k @ x1[:, i_k(n)])   (gather source is
   x1[:, 0:64], the reference's matrix[:, ind] quirk)
  selection via one-hot matmuls: out^T = sum_k Y'_k^T @ onehot_k

Precision scheme: x and candidates are split hi/lo into bf16 pairs on the
host (16-bit effective mantissa); the distance matmul streams
[2ch | 2cl] against stationary [xh ; xl], accumulating both halves into
the same PSUM columns via a stride-0 (broadcast) out AP, on top of a
-s2 prefill (rank-2 ones matmul with hi/lo s2 rows).  Measured end-to-end
rel err of this scheme ~4e-3 (gate 2e-2); ranking wrong-pick rate ~0.

Sharding: pure data parallel, 2 batches per core on 8 cores.
"""

import numpy as np

B, C, H, W = 16, 64, 96, 96
N = H * W            # 9216
S2 = 64              # candidate count (8x8 grid)
KNN = 3
NCORES = 8
BPC = B // NCORES    # batches per core = 2
NCHUNK = 128         # n per distance-matmul chunk
MACRO = 1024         # n per macro tile (8 chunks)
NMACRO = N // MACRO  # 9
PSC = 512            # psum score tile cols (4 chunks)

_CACHE = {}


def _build():
    import concourse.bass as bass
    import concourse.bacc as bacc
    import concourse.mybir as mybir
    import concourse.tile as tile
    from contextlib import ExitStack

    f32 = mybir.dt.float32
    bf16 = mybir.dt.bfloat16
    f16 = mybir.dt.float16
    u16 = mybir.dt.uint16
    AF = mybir.ActivationFunctionType

    nc = bacc.Bacc()
    xhl_h = nc.declare_dram_parameter("xhl", [BPC, 2 * C, N], bf16, isOutput=False)
    # consts f32: [0:128]=identity(f32, for idx transpose), 128=bias, 129=iotacol
    consts_h = nc.declare_dram_parameter("consts", [128, 130], f32, isOutput=False)
    # consts2 bf16: cols 0:192 = wstA ([Wh;Wh] per k), 192:384 = wstB
    # ([Wl;Wl]), 384:512 = rcand b0, 512:640 = rcand b1,
    # 640:1152 = s2rows (b0 rows 0:2, b1 rows 2:4)
    consts2_h = nc.declare_dram_parameter("consts2", [128, 1152], bf16, isOutput=False)
    out_h = nc.declare_dram_parameter("out", [BPC, C, N], f32, isOutput=True)

    with tile.TileContext(nc) as tc, ExitStack() as ctx:
        const_p = ctx.enter_context(tc.tile_pool(name="const", bufs=1))
        xin_p = ctx.enter_context(tc.tile_pool(name="xin", bufs=2))
        small_p = ctx.enter_context(tc.tile_pool(name="small", bufs=2))
        idx_p = ctx.enter_context(tc.tile_pool(name="idx", bufs=3))
        ssb_p = ctx.enter_context(tc.tile_pool(name="ssb", bufs=4))
        bcast_p = ctx.enter_context(tc.tile_pool(name="bcast", bufs=3))
        oh_p = ctx.enter_context(tc.tile_pool(name="oh", bufs=3))
        osb_p = ctx.enter_context(tc.tile_pool(name="osb", bufs=3))
        dram_p = ctx.enter_context(tc.tile_pool(name="bounce", bufs=3, space="DRAM"))
        ps_nk = ctx.enter_context(tc.tile_pool(name="ps_nk", bufs=3, space="PSUM"))
        ps_it = ctx.enter_context(tc.tile_pool(name="ps_it", bufs=1, space="PSUM"))
        ps_out = ctx.enter_context(tc.tile_pool(name="ps_out", bufs=1, space="PSUM"))
        ps_misc = ctx.enter_context(tc.tile_pool(name="ps_misc", bufs=1, space="PSUM"))

        consts_sb = const_p.tile([128, 130], f32)
        nc.sync.dma_start(consts_sb[:], consts_h.ap())
        identf_sb = consts_sb[:, 0:128]
        bias_sb = consts_sb[0:C, 128:129]
        iotacol_sb = consts_sb[:, 129:130]

        consts2_sb = const_p.tile([128, 1152], bf16)
        nc.sync.dma_start(consts2_sb[:], consts2_h.ap())

        ones2_sb = const_p.tile([66, 128], bf16, tag="ones2")
        nc.vector.memset(ones2_sb[:], 1.0)

        for b in range(BPC):
            # ---- x hi/lo [128, N] bf16 ----
            xhl = xin_p.tile([2 * C, N], bf16, tag="xhl")
            nc.sync.dma_start(xhl[:], xhl_h.ap()[b])

            rc_sb = consts2_sb[:, 384 + 128 * b : 384 + 128 * (b + 1)]
            s2row_sb = consts2_sb[64 * b : 64 * b + 2, 640:1152]
            ones2b_sb = ones2_sb[64 * b : 64 * b + 2, :]

            # ---- Y' tables: Y'_k[s,o] = sum_c x1[c, s]*W_k[o, c] ----
            # two passes: (xh+xl)Wh then += (xh+xl)Wl
            yall = ps_misc.tile([128, 128], f32, tag="misc")
            wstA = consts2_sb[:, 0:192]
            wstB = consts2_sb[:, 192:384]
            x64 = xhl[:, 0:S2]
            nc.tensor.matmul(yall[0:64, 0:64], x64, wstA[:, 0:64], start=True, stop=False)
            nc.tensor.matmul(yall[0:64, 0:64], x64, wstB[:, 0:64], start=False, stop=True)
            nc.tensor.matmul(yall[64:128, 0:64], x64, wstA[:, 64:128], start=True, stop=False)
            nc.tensor.matmul(yall[64:128, 0:64], x64, wstB[:, 64:128], start=False, stop=True)
            nc.tensor.matmul(yall[0:64, 64:128], x64, wstA[:, 128:192], start=True, stop=False)
            nc.tensor.matmul(yall[0:64, 64:128], x64, wstB[:, 128:192], start=False, stop=True)
            ysel01 = small_p.tile([128, C], f16, tag="ysel01")
            nc.vector.tensor_copy(ysel01[:], yall[:, 0:64])
            ysel2 = small_p.tile([C, C], f16, tag="ysel2")
            nc.vector.tensor_copy(ysel2[:], yall[0:64, 64:128])

            # ---- main loop over macro tiles (selection software-pipelined
            # one macro behind distance, to keep the in-order PE queue busy) ----
            def emit_selection(oh01, oh2, n0):
                po = ps_out.tile([64, MACRO], f32, tag="po")
                for h in range(2):
                    ph = po[:, h * 512 : (h + 1) * 512]
                    nc.tensor.matmul(ph, ysel01[:], oh01[:, h * 512 : (h + 1) * 512], start=True, stop=False)
                    nc.tensor.matmul(ph, ysel2[:], oh2[:, h * 512 : (h + 1) * 512], start=False, stop=True)
                osb = osb_p.tile([64, MACRO], f32, tag="osb")
                nc.scalar.activation(osb[:], po[:], AF.Relu, bias=bias_sb)
                nc.sync.dma_start(out_h.ap()[b][:, n0 : n0 + MACRO], osb[:])

            prev = None
            for m in range(NMACRO):
                n0 = m * MACRO
                idx_all = idx_p.tile([NCHUNK, 8, 8], u16, tag="idx")
                pnk = ps_nk.tile([NCHUNK, PSC], f32, tag="nk")
                # -s2 prefill of all 8 chunk-blocks (hi+lo rows)
                nc.tensor.matmul(pnk[:], ones2b_sb, s2row_sb[:], start=True, stop=False)
                for c8 in range(8):
                    sl = pnk[:, c8 * S2 : (c8 + 1) * S2]
                    xch = xhl[:, n0 + c8 * NCHUNK : n0 + (c8 + 1) * NCHUNK]
                    # (xh+xl)*2ch then += (xh+xl)*2cl, on top of -s2 prefill
                    nc.tensor.matmul(sl, xch, rc_sb[:, 0:S2], start=False, stop=False)
                    nc.tensor.matmul(sl, xch, rc_sb[:, S2:128], start=False, stop=True)
                for hh in range(2):
                    ssb = ssb_p.tile([NCHUNK, PSC // 2], f32, tag="ssb")
                    nc.scalar.copy(ssb[:], pnk[:, hh * 256 : (hh + 1) * 256])
                    for c4 in range(4):
                        c8 = hh * 4 + c4
                        ssl = ssb[:, c4 * S2 : (c4 + 1) * S2]
                        maxv = idx_p.tile([NCHUNK, 8], f32, tag="maxv")
                        nc.vector.max(out=maxv[:], in_=ssl)
                        nc.vector.max_index(out=idx_all[:, :, c8], in_max=maxv[:], in_values=ssl)

                # transpose idx (rank-major rows) via PE -> bounce -> bcast
                idxf = idx_p.tile([NCHUNK, 64], f32, tag="idxf")
                nc.vector.tensor_copy(idxf[:], idx_all[:].rearrange("p a b -> p (a b)"))
                pit = ps_it.tile([64, NCHUNK], f32, tag="it")
                nc.tensor.transpose(pit[:], idxf[:], identf_sb[:])
                idxt_sb = idx_p.tile([KNN * 8, NCHUNK], u16, tag="idxt_sb")
                nc.vector.tensor_copy(idxt_sb[:], pit[0 : KNN * 8, :])  # f32 -> u16
                dbt = dram_p.tile([KNN, MACRO], u16, tag="dbt")
                nc.sync.dma_start(dbt[:], idxt_sb[:])
                idxb01 = bcast_p.tile([128, MACRO], u16, tag="idxb01")
                idxb2 = bcast_p.tile([64, MACRO], u16, tag="idxb2")
                nc.sync.dma_start(idxb01[0:64, :], dbt[0:1, :].to_broadcast((64, MACRO)))
                nc.sync.dma_start(idxb01[64:128, :], dbt[1:2, :].to_broadcast((64, MACRO)))
                nc.sync.dma_start(idxb2[:], dbt[2:3, :].to_broadcast((64, MACRO)))

                # one-hots (fp16): oh[s, n] = [s == i_k(n)]
                oh01 = oh_p.tile([128, MACRO], f16, tag="oh01")
                oh2 = oh_p.tile([64, MACRO], f16, tag="oh2")
                nc.vector.tensor_scalar(
                    oh01[:], idxb01[:], iotacol_sb, None,
                    op0=mybir.AluOpType.is_equal,
                )
                nc.vector.tensor_scalar(
                    oh2[:], idxb2[:], iotacol_sb[0:64, :], None,
                    op0=mybir.AluOpType.is_equal,
                )

                emit_selection(oh01, oh2, n0)

    nc.compile()
    return nc


def _host_inputs(x, conv_w, conv_b):
    """Build all per-core DRAM inputs (host prep: dtype split + packing)."""
    import ml_dtypes
    bf = ml_dtypes.bfloat16

    def split(v):
        hi = v.astype(bf)
        lo = (v - hi.astype(np.float32)).astype(bf)
        return hi, lo

    xr = x.reshape(B, C, N)
    # sample grid (static): index s = 8i+j over rounded linspace
    xi = np.round(np.linspace(0, H - 1, 8)).astype(np.int32)
    yi = np.round(np.linspace(0, W - 1, 8)).astype(np.int32)
    xg, yg = np.meshgrid(xi, yi, indexing="ij")
    gidx = (xg * W + yg).reshape(-1)

    consts = np.zeros((128, 130), np.float32)
    consts[:, 0:128] = np.eye(128, dtype=np.float32)
    consts[0:C, 128] = conv_b
    consts[:, 129] = np.arange(128, dtype=np.float32) % 64

    wst = np.zeros((128, 384), bf)
    for k in range(KNN):
        wh, wl = split(conv_w[:, :, k].T)     # [c, o]
        wst[0:C, k * 64 : (k + 1) * 64] = wh
        wst[C:, k * 64 : (k + 1) * 64] = wh
        wst[0:C, 192 + k * 64 : 192 + (k + 1) * 64] = wl
        wst[C:, 192 + k * 64 : 192 + (k + 1) * 64] = wl

    in_maps = []
    for core in range(NCORES):
        consts2 = np.zeros((128, 1152), bf)
        consts2[:, 0:384] = wst
        xhl_core = np.empty((BPC, 2 * C, N), bf)
        for b in range(BPC):
            x1 = xr[core * BPC + b]          # [C, N] f32
            xh, xl = split(x1)
            xhl_core[b, 0:C] = xh
            xhl_core[b, C:] = xl
            cand = x1[:, gidx]               # [C, 64]
            ch, cl = split(2.0 * cand)
            base = 384 + 128 * b
            consts2[0:C, base : base + 64] = ch
            consts2[C:, base : base + 64] = ch
            consts2[0:C, base + 64 : base + 128] = cl
            consts2[C:, base + 64 : base + 128] = cl
            s2 = (cand.astype(np.float64) ** 2).sum(0).astype(np.float32)
            s2h, s2l = split(-s2)
            consts2[64 * b, 640:1152] = np.tile(s2h, 8)
            consts2[64 * b + 1, 640:1152] = np.tile(s2l, 8)
        in_maps.append({
            "xhl": xhl_core,
            "consts": consts,
            "consts2": consts2,
        })
    return in_maps


def kernel(x, conv_w, conv_b):
    from concourse.bass_utils import run_bass_kernel_spmd

    x = np.ascontiguousarray(np.asarray(x, dtype=np.float32))
    conv_w = np.asarray(conv_w, dtype=np.float32)
    conv_b = np.asarray(conv_b, dtype=np.float32)

    if "nc" not in _CACHE:
        _CACHE["nc"] = _build()
    nc = _CACHE["nc"]

    in_maps = _host_inputs(x, conv_w, conv_b)
    res = run_bass_kernel_spmd(nc, in_maps, list(range(NCORES))).results
    out = np.empty((B, C, N), np.float32)
    for core in range(NCORES):
        out[core * BPC : (core + 1) * BPC] = res[core]["out"].reshape(BPC, C, N)
    return out.reshape(B, C, H, W)
